# revision 7
# baseline (speedup 1.0000x reference)
"""Trainium2 Bass kernel for nn_ConvProjector (conv3x3 -> ReLU -> conv3x3 -> ReLU
-> adaptive-avg-pool upsample 32x32 -> 687x1024 -> 1x1 conv 256->24 + bias).

v1 structure (vs v0 baseline):
  * 3 DMA queues (sync HWDGE, scalar HWDGE, gpsimd SWDGE) for weight
    streaming and output writes.
  * conv1 bias + out-of-image row masking folded into the matmul via an
    indicator channel in the spare kc4 partition (row 64): psum already
    holds conv+bias, and pad rows come out exactly 0 after plain ReLU.
  * 1x1-conv bias folded into the W-expansion matmul via a 33rd lhsT
    partition holding br per output column.
  * Expanded tile laid out [pure | pure | avg] (48 x 3072 per half) so the
    H-replication DMA uses 4KB contiguous descriptors (2 output rows per
    descriptor); rows 20-21 ship as one 4KB-desc DMA from offset 1024.
  * conv2 + 1x1 + expansion + store split into two halves (h2 rows 0-2 /
    3-4) so half B's compute overlaps half A's output DMA.
  * Sharding: unchanged - core k owns input rows 4k..4k+3, channel-complete
    convs on a 9-row x slice, no collectives.
"""
import sys

if '/opt/trn_rl_repo' not in sys.path:
    sys.path.insert(0, '/opt/trn_rl_repo')

import numpy as np

IN_C, MID_C, OUT_C = 576, 256, 24
H = W = 32
OUT_H, OUT_W = 687, 1024
NCORES = 8
P = 128
KC1 = 5           # ceil(576/128) input-channel chunks for conv1 (padded to 640)
KC2 = 2           # 256/128 chunks for conv2 / 1x1
MC = 2            # 256/128 output-channel chunks for conv1/conv2
W36 = 36          # padded row width (2 zero cols each side)
RX, R1, R2 = 9, 7, 5          # x rows / h1 rows / h2 (=r) rows per core
XBLK = RX * W36               # 324  per-kc x block
XSLACK = 16                   # rhs overrun slack
N1 = 256                      # conv1 matmul N (valid span 252)
H1BLK = R1 * W36              # 252  per-mc h1 block
H1SLACK = 80
N2A = 112                     # conv2 half-A matmul N (h2 rows 0-2, valid 108)
N2B = 76                      # conv2 half-B matmul N (h2 rows 3-4, valid 72)
RUN = 22                      # output rows per owned input row

_prog_cache = {}


def _h_runs():
    i = np.arange(OUT_H)
    s = (i * H) // OUT_H
    t = np.searchsorted(s, np.arange(H + 1), side='left')
    return s, t


def _build_program():
    import concourse.bass as bass
    import concourse.bacc as bacc
    import concourse.mybir as mybir
    from concourse.tile import TileContext

    f32 = mybir.dt.float32
    f16 = mybir.dt.float16
    nc = bacc.Bacc("TRN2", target_bir_lowering=False, debug=False,
                   num_devices=NCORES)

    xs_d = nc.dram_tensor("xs", [P, KC1 * XBLK + XSLACK], f16, kind="ExternalInput")
    w1_d = nc.dram_tensor("w1p", [P, 9 * KC1 * MC * P], f16, kind="ExternalInput")
    w2_d = nc.dram_tensor("w2p", [P, 9 * KC2 * MC * P], f16, kind="ExternalInput")
    wr_d = nc.dram_tensor("wrp", [P, KC2 * OUT_C], f16, kind="ExternalInput")
    b2_d = nc.dram_tensor("b2p", [P, MC], f32, kind="ExternalInput")
    em_d = nc.dram_tensor("emp", [33, 2048], f16, kind="ExternalInput")
    br_d = nc.dram_tensor("browp", [1, 120], f16, kind="ExternalInput")
    out_d = nc.dram_tensor("outb", [96, RUN * OUT_W], f16, kind="ExternalOutput")

    Relu = mybir.ActivationFunctionType.Relu
    Ident = mybir.ActivationFunctionType.Identity
    w1blk = KC1 * MC * P          # 1280 elems per tap row
    w2blk = 3 * KC2 * MC * P      # 1536 elems per 3-tap group row

    with TileContext(nc) as tc:
        with (
            tc.tile_pool(name="sb", bufs=1) as sb,
            tc.tile_pool(name="ps", bufs=1, space="PSUM") as psp,
        ):
            x_t = sb.tile([P, KC1 * XBLK + XSLACK], f16)
            w1_ts = [sb.tile([P, w1blk], f16, tag=f"w1_{t}", name=f"w1t{t}")
                     for t in range(9)]
            w2_ts = [sb.tile([P, w2blk], f16, tag=f"w2_{t}", name=f"w2t{t}")
                     for t in range(3)]
            wr_t = sb.tile([P, KC2 * OUT_C], f16)
            b2_t = sb.tile([P, MC], f32)
            em_t = sb.tile([33, 2048], f16)
            h1_t = sb.tile([P, MC * H1BLK + H1SLACK], f16)
            h2a_t = sb.tile([P, MC * 96], f16)
            h2b_t = sb.tile([P, MC * 64], f16)
            rt_t = sb.tile([33, 120], f16)
            rwa_t = sb.tile([48, 3072], f16)
            rwb_t = sb.tile([48, 3072], f16)

            # ---- input loads: 3 queues, taps arrive roughly in order ----
            def tap_dma(eng, t):
                eng.dma_start(
                    w1_ts[t][:],
                    bass.AP(w1_d, t * w1blk, [[9 * w1blk, P], [1, w1blk]]))

            def w2_dma(eng, t):
                eng.dma_start(
                    w2_ts[t][:],
                    bass.AP(w2_d, t * w2blk, [[3 * w2blk, P], [1, w2blk]]))

            # gpsimd: t0 first so conv1 starts earliest
            tap_dma(nc.gpsimd, 0)
            nc.sync.dma_start(x_t[:], xs_d.ap())
            tap_dma(nc.scalar, 1)
            tap_dma(nc.sync, 2)
            tap_dma(nc.gpsimd, 3)
            tap_dma(nc.scalar, 4)
            tap_dma(nc.sync, 5)
            tap_dma(nc.gpsimd, 6)
            tap_dma(nc.scalar, 7)
            tap_dma(nc.sync, 8)
            w2_dma(nc.gpsimd, 0)
            w2_dma(nc.scalar, 1)
            w2_dma(nc.sync, 2)
            nc.scalar.dma_start(wr_t[:], wr_d.ap())
            nc.scalar.dma_start(b2_t[:], b2_d.ap())
            nc.sync.dma_start(em_t[:], em_d.ap())
            nc.gpsimd.dma_start(rt_t[32:33, :], br_d.ap())

            # h1 pad cols must be zero; ReLU only writes valid 32-col spans.
            nc.vector.memset(h1_t[:], 0.0)

            # ---- conv1: 576 -> 256 over 7 rows, bias via indicator ------
            ps1s = [psp.tile([P, N1], f32, tag="cva", name="ps1a"),
                    psp.tile([P, N1], f32, tag="cvb", name="ps1b")]
            n_acc = 9 * KC1
            i_acc = 0
            for tap in range(9):
                ky, kx = tap // 3, tap % 3
                off = ky * W36 + kx + 1
                for kc in range(KC1):
                    for mc in range(MC):
                        nc.tensor.matmul(
                            ps1s[mc][:, :],
                            lhsT=w1_ts[tap][:, (kc * MC + mc) * P:
                                            (kc * MC + mc) * P + P],
                            rhs=x_t[:, kc * XBLK + off: kc * XBLK + off + N1],
                            start=(i_acc == 0), stop=(i_acc == n_acc - 1),
                        )
                    i_acc += 1
            for mc in range(MC):
                ps1 = ps1s[mc]
                src = bass.AP(ps1.tensor, ps1.offset,
                              [[N1, P], [W36, R1], [1, 32]])
                dstb = h1_t[:, :]
                dst = bass.AP(dstb.tensor, dstb.offset + mc * H1BLK + 2,
                              [[MC * H1BLK + H1SLACK, P], [W36, R1], [1, 32]])
                nc.scalar.activation(dst, src, Relu)

            # ---- per-half pipeline: conv2 -> 1x1 -> expand -> store -----
            psr = psp.tile([32, 120], f32, tag="psr")
            psrb = psr[:, :]
            rtb = rt_t[:, :]

            def half(idx, rows, n2, row0, h2_t, rw_t, engs):
                nacc2 = 9 * KC2
                iacc2 = 0
                ps2s = [psp.tile([P, N1], f32, tag="cva", name=f"ps2a{idx}"),
                        psp.tile([P, N1], f32, tag="cvb", name=f"ps2b{idx}")]
                for tap in range(9):
                    ky, kx = tap // 3, tap % 3
                    off = ky * W36 + kx + 1 + row0 * W36
                    for kc in range(KC2):
                        for mc in range(MC):
                            nc.tensor.matmul(
                                ps2s[mc][:, 0:n2],
                                lhsT=w2_ts[tap // 3][:,
                                    ((tap % 3) * KC2 + kc) * MC * P + mc * P:
                                    ((tap % 3) * KC2 + kc) * MC * P + mc * P + P],
                                rhs=h1_t[:, kc * H1BLK + off:
                                         kc * H1BLK + off + n2],
                                start=(iacc2 == 0), stop=(iacc2 == nacc2 - 1),
                            )
                        iacc2 += 1
                for mc in range(MC):
                    ps2 = ps2s[mc]
                    src2 = bass.AP(ps2.tensor, ps2.offset,
                                   [[N1, P], [W36, rows], [1, 32]])
                    h2b_ = h2_t[:, :]
                    dst2 = bass.AP(h2b_.tensor, h2b_.offset + mc * rows * 32,
                                   [[MC * rows * 32, P], [32, rows], [1, 32]])
                    nc.scalar.activation(dst2, src2, Relu,
                                         bias=b2_t[:, mc:mc + 1])

                # 1x1 conv -> psr cols (24r + c), r-major contiguous
                for r in range(rows):
                    for kc in range(KC2):
                        nc.tensor.matmul(
                            psr[:, (row0 + r) * OUT_C:(row0 + r + 1) * OUT_C],
                            lhsT=h2_t[:, kc * rows * 32 + r * 32:
                                      kc * rows * 32 + r * 32 + 32],
                            rhs=wr_t[:, kc * OUT_C:(kc + 1) * OUT_C],
                            start=(kc == 0), stop=(kc == KC2 - 1),
                        )
                # psr -> rt (fp32 -> fp16), same (24r + c) layout
                nc.vector.tensor_copy(
                    bass.AP(rtb.tensor, rtb.offset + row0 * OUT_C,
                            [[120, 32], [1, rows * OUT_C]]),
                    bass.AP(psrb.tensor, psrb.offset + row0 * OUT_C,
                            [[120, 32], [1, rows * OUT_C]]))

                # ---- W expansion, bias via rt row 32 --------------------
                g0 = 2 * idx  # first output group of this half
                psw = psp.tile([48, 1024], f32, tag="psw", name=f"psw{idx}")
                psa = psp.tile([48, 1024], f32, tag="psa", name=f"psa{idx}")
                lhs_p = bass.AP(rtb.tensor, rtb.offset + g0 * OUT_C,
                                [[120, 33], [1, 48]])
                lhs_n = bass.AP(rtb.tensor, rtb.offset + (g0 + 1) * OUT_C,
                                [[120, 32], [1, 48]])
                for j in range(2):
                    nc.tensor.matmul(psw[:, j * 512:(j + 1) * 512],
                                     lhsT=lhs_p,
                                     rhs=em_t[:, j * 512:(j + 1) * 512],
                                     start=True, stop=True)
                    nc.tensor.matmul(psa[:, j * 512:(j + 1) * 512],
                                     lhsT=lhs_p,
                                     rhs=em_t[:, 1024 + j * 512:
                                              1024 + (j + 1) * 512],
                                     start=True, stop=False)
                    nc.tensor.matmul(psa[:, j * 512:(j + 1) * 512],
                                     lhsT=lhs_n,
                                     rhs=em_t[0:32, 1024 + j * 512:
                                              1024 + (j + 1) * 512],
                                     start=False, stop=True)
                # PSUM -> SBUF: [pure | pure | avg]
                nc.scalar.activation(rw_t[:, 0:1024], psw[:, :], Ident)
                nc.vector.tensor_copy(rw_t[:, 1024:2048], psw[:, :])
                nc.vector.tensor_copy(rw_t[:, 2048:3072], psa[:, :])

                # ---- store: rows 0-19 as 2-row 4KB descs, rows 20-21 ----
                rwb_ = rw_t[:, :]
                base = idx * 48 * RUN * OUT_W
                e0, e1, e2 = engs
                e0.dma_start(
                    bass.AP(out_d, base, [[RUN * OUT_W, 48], [2048, 5], [1, 2048]]),
                    bass.AP(rwb_.tensor, rwb_.offset, [[3072, 48], [0, 5], [1, 2048]]))
                e1.dma_start(
                    bass.AP(out_d, base + 10 * OUT_W,
                            [[RUN * OUT_W, 48], [2048, 5], [1, 2048]]),
                    bass.AP(rwb_.tensor, rwb_.offset, [[3072, 48], [0, 5], [1, 2048]]))
                e2.dma_start(
                    bass.AP(out_d, base + 20 * OUT_W, [[RUN * OUT_W, 48], [1, 2048]]),
                    bass.AP(rwb_.tensor, rwb_.offset + 1024, [[3072, 48], [1, 2048]]))

            half(0, 3, N2A, 0, h2a_t, rwa_t, (nc.sync, nc.scalar, nc.gpsimd))
            half(1, 2, N2B, 3, h2b_t, rwb_t, (nc.gpsimd, nc.sync, nc.scalar))

    nc.compile()
    return nc


def _pack_inputs(x, w1, b1, w2, b2, wr, br):
    x = np.asarray(x, np.float32)
    w1 = np.asarray(w1, np.float32)
    w2 = np.asarray(w2, np.float32)
    wr = np.asarray(wr, np.float32)
    b1 = np.asarray(b1, np.float32)
    b2 = np.asarray(b2, np.float32)
    br = np.asarray(br, np.float32)

    xp = np.zeros((NCORES, P, KC1, RX, W36), np.float16)
    xv = x[0]  # (576, 32, 32)
    for k in range(NCORES):
        for r in range(RX):
            g = 4 * k - 2 + r
            if 0 <= g < H:
                blkv = xv[:, g, :]  # (576, 32)
                xp[k, :, :4, r, 2:34] = blkv[:512].reshape(4, P, W).transpose(1, 0, 2)
                xp[k, :64, 4, r, 2:34] = blkv[512:]
                xp[k, 64, 4, r, 2:34] = 1.0  # valid-row indicator (bias)
            else:
                # invalid-row indicator: forces h1 row r-1 below zero so
                # ReLU clamps it to exactly 0 (replaces the row mask)
                xp[k, 65, 4, r, 2:34] = 1.0
    xp = xp.reshape(NCORES, P, KC1 * XBLK)
    xp = np.concatenate([xp, np.zeros((NCORES, P, XSLACK), np.float16)], axis=2)

    # w1: [p, tap, kc, mc, m] = w1[mc*128+m, kc*128+p, ky, kx]
    w1p = np.zeros((P, 9, KC1, MC, P), np.float16)
    w1v = w1.transpose(2, 3, 1, 0).reshape(9, IN_C, MID_C)  # (tap, ci, co)
    w1p[:, :, :4, :, :] = (
        w1v[:, :512, :].reshape(9, 4, P, MC, P).transpose(2, 0, 1, 3, 4))
    w1p[:64, :, 4, :, :] = w1v[:, 512:, :].reshape(9, 64, MC, P).transpose(1, 0, 2, 3)
    # bias enters via the indicator channel on the center tap; the
    # invalid-row indicator drives out-of-image h1 rows to relu(-1000)=0
    w1p[64, 4, 4, :, :] = b1.reshape(MC, P)
    w1p[65, 4, 4, :, :] = -1000.0
    w1p = w1p.reshape(P, 9 * KC1 * MC * P)

    w2p = np.zeros((P, 9, KC2, MC, P), np.float16)
    w2v = w2.transpose(2, 3, 1, 0).reshape(9, MID_C, MID_C)
    w2p[:, :, :, :, :] = (
        w2v.reshape(9, KC2, P, MC, P).transpose(2, 0, 1, 3, 4))
    w2p = w2p.reshape(P, 9 * KC2 * MC * P)

    wrp = wr.T.reshape(KC2, P, OUT_C).transpose(1, 0, 2).reshape(P, KC2 * OUT_C)
    wrp = np.ascontiguousarray(wrp, np.float16)
    b2p = b2.reshape(MC, P).T.copy()

    # em: [33, 2048]: cols 0-1023 = E with bias row 1, cols 1024-2047 = E/2
    # with bias row 1 (bias added once; the lhs_n matmul excludes row 32)
    E = (np.arange(OUT_W) // 32 == np.arange(32)[:, None]).astype(np.float16)
    em = np.zeros((33, 2048), np.float16)
    em[:32, :1024] = E
    em[:32, 1024:] = 0.5 * E
    em[32, :] = 1.0

    # bias row for rt (free idx 24r + c -> br[c])
    brow = np.tile(br, 5).reshape(1, 120).astype(np.float16)

    shared = dict(w1p=w1p, w2p=w2p, wrp=wrp, b2p=b2p, emp=em, browp=brow)
    in_maps = []
    for k in range(NCORES):
        m = dict(shared)
        m["xs"] = np.ascontiguousarray(xp[k])
        in_maps.append(m)
    return in_maps


def kernel(x, w1, b1, w2, b2, wr, br):
    from concourse.bass_utils import run_bass_kernel_spmd

    if "nc" not in _prog_cache:
        _prog_cache["nc"] = _build_program()
    nc = _prog_cache["nc"]

    in_maps = _pack_inputs(x, w1, b1, w2, b2, wr, br)
    res = run_bass_kernel_spmd(nc, in_maps, list(range(NCORES)))

    _, t = _h_runs()
    out = np.empty((1, OUT_C, OUT_H, OUT_W), np.float32)
    for k in range(NCORES):
        # outb rows: m = half*48 + h'*24 + c ; cols: RUN x 1024
        buf = res.results[k]["outb"].astype(np.float32)
        buf = buf.reshape(2, 2, OUT_C, RUN, OUT_W)
        for hl in range(4):
            h = 4 * k + hl
            n = t[h + 1] - t[h]
            g = buf[hl // 2, hl % 2]  # (OUT_C, RUN, OUT_W)
            out[0, :, t[h]:t[h] + n - 1, :] = g[:, 0:n - 1, :]
            # last row of the run: averaged row (except global last row,
            # where the run is pure replication and n-1 <= 20)
            if h < H - 1:
                out[0, :, t[h] + n - 1, :] = g[:, RUN - 1, :]
            else:
                out[0, :, t[h] + n - 1, :] = g[:, n - 1, :]
    return out


# revision 12
# speedup vs baseline: 1.1590x; 1.1590x over previous
"""Trainium2 Bass kernel for nn_ConvProjector (conv3x3 -> ReLU -> conv3x3 -> ReLU
-> adaptive-avg-pool upsample 32x32 -> 687x1024 -> 1x1 conv 256->24 + bias).

v1 structure (vs v0 baseline):
  * 3 DMA queues (sync HWDGE, scalar HWDGE, gpsimd SWDGE) for weight
    streaming and output writes.
  * conv1 bias + out-of-image row masking folded into the matmul via an
    indicator channel in the spare kc4 partition (row 64): psum already
    holds conv+bias, and pad rows come out exactly 0 after plain ReLU.
  * 1x1-conv bias folded into the W-expansion matmul via a 33rd lhsT
    partition holding br per output column.
  * Expanded tile laid out [pure | pure | avg] (48 x 3072 per half) so the
    H-replication DMA uses 4KB contiguous descriptors (2 output rows per
    descriptor); rows 20-21 ship as one 4KB-desc DMA from offset 1024.
  * conv2 + 1x1 + expansion + store split into two halves (h2 rows 0-2 /
    3-4) so half B's compute overlaps half A's output DMA.
  * Sharding: unchanged - core k owns input rows 4k..4k+3, channel-complete
    convs on a 9-row x slice, no collectives.
"""
import sys

if '/opt/trn_rl_repo' not in sys.path:
    sys.path.insert(0, '/opt/trn_rl_repo')

import numpy as np

IN_C, MID_C, OUT_C = 576, 256, 24
H = W = 32
OUT_H, OUT_W = 687, 1024
NCORES = 8
P = 128
KC1 = 5           # ceil(576/128) input-channel chunks for conv1 (padded to 640)
KC2 = 2           # 256/128 chunks for conv2 / 1x1
MC = 2            # 256/128 output-channel chunks for conv1/conv2
W36 = 36          # padded row width (2 zero cols each side)
RX, R1, R2 = 9, 7, 5          # x rows / h1 rows / h2 (=r) rows per core
XBLK = RX * W36               # 324  per-kc x block
XSLACK = 16                   # rhs overrun slack
N1 = 256                      # conv1 matmul N (valid span 252)
H1BLK = R1 * W36              # 252  per-mc h1 block
H1SLACK = 80
N2 = 192                      # conv2 matmul N (valid span 176)
RUN = 22                      # output rows per owned input row

_prog_cache = {}


def _h_runs():
    i = np.arange(OUT_H)
    s = (i * H) // OUT_H
    t = np.searchsorted(s, np.arange(H + 1), side='left')
    return s, t


def _build_program():
    import concourse.bass as bass
    import concourse.bacc as bacc
    import concourse.mybir as mybir
    from concourse.tile import TileContext

    f32 = mybir.dt.float32
    f16 = mybir.dt.float16
    nc = bacc.Bacc("TRN2", target_bir_lowering=False, debug=False,
                   num_devices=NCORES)

    xs_d = nc.dram_tensor("xs", [P, KC1 * XBLK + XSLACK], f16, kind="ExternalInput")
    w1_d = nc.dram_tensor("w1p", [P, 9 * KC1 * MC * P], f16, kind="ExternalInput")
    w2_d = nc.dram_tensor("w2p", [P, 9 * KC2 * MC * P], f16, kind="ExternalInput")
    wr_d = nc.dram_tensor("wrp", [P, KC2 * OUT_C], f16, kind="ExternalInput")
    b2_d = nc.dram_tensor("b2p", [P, MC], f32, kind="ExternalInput")
    em_d = nc.dram_tensor("emp", [33, 2048], f16, kind="ExternalInput")
    br_d = nc.dram_tensor("browp", [1, 120], f16, kind="ExternalInput")
    out_d = nc.dram_tensor("outb", [96, RUN * OUT_W], f16, kind="ExternalOutput")

    Relu = mybir.ActivationFunctionType.Relu
    Ident = mybir.ActivationFunctionType.Identity
    w1blk = KC1 * MC * P          # 1280 elems per tap row
    w2blk = 3 * KC2 * MC * P      # 1536 elems per 3-tap group row

    with TileContext(nc) as tc:
        with (
            tc.tile_pool(name="sb", bufs=1) as sb,
            tc.tile_pool(name="ps", bufs=1, space="PSUM") as psp,
        ):
            x_t = sb.tile([P, KC1 * XBLK + XSLACK], f16)
            w1_ts = [sb.tile([P, w1blk], f16, tag=f"w1_{t}", name=f"w1t{t}")
                     for t in range(9)]
            w2_ts = [sb.tile([P, w2blk], f16, tag=f"w2_{t}", name=f"w2t{t}")
                     for t in range(3)]
            wr_t = sb.tile([P, KC2 * OUT_C], f16)
            b2_t = sb.tile([P, MC], f32)
            em_t = sb.tile([33, 2048], f16)
            h1_t = sb.tile([P, MC * H1BLK + H1SLACK], f16)
            h2_t = sb.tile([P, MC * 160], f16)
            rt_t = sb.tile([33, 120], f16)
            rw_t = sb.tile([96, 3072], f16)

            # ---- input loads: 3 queues, taps arrive roughly in order ----
            def tap_dma(eng, t):
                eng.dma_start(
                    w1_ts[t][:],
                    bass.AP(w1_d, t * w1blk, [[9 * w1blk, P], [1, w1blk]]))

            def w2_dma(eng, t):
                eng.dma_start(
                    w2_ts[t][:],
                    bass.AP(w2_d, t * w2blk, [[3 * w2blk, P], [1, w2blk]]))

            # t0 first on the fast HWDGE queue; x in parallel on scalar
            tap_dma(nc.sync, 0)
            nc.scalar.dma_start(x_t[:], xs_d.ap())
            tap_dma(nc.gpsimd, 2)
            tap_dma(nc.scalar, 1)
            tap_dma(nc.sync, 3)
            tap_dma(nc.gpsimd, 5)
            tap_dma(nc.scalar, 4)
            tap_dma(nc.sync, 6)
            tap_dma(nc.gpsimd, 8)
            tap_dma(nc.scalar, 7)
            w2_dma(nc.sync, 0)
            w2_dma(nc.scalar, 1)
            w2_dma(nc.gpsimd, 2)
            nc.scalar.dma_start(wr_t[:], wr_d.ap())
            nc.scalar.dma_start(b2_t[:], b2_d.ap())
            nc.sync.dma_start(em_t[:], em_d.ap())
            nc.gpsimd.dma_start(rt_t[32:33, :], br_d.ap())

            # h1 pad cols must be zero; ReLU only writes valid 32-col spans.
            nc.vector.memset(h1_t[:], 0.0)

            # ---- conv1: 576 -> 256 over 7 rows, bias via indicator ------
            ps1s = [psp.tile([P, N1], f32, tag="cva", name="ps1a"),
                    psp.tile([P, N1], f32, tag="cvb", name="ps1b")]
            n_acc = 9 * KC1
            i_acc = 0
            for tap in range(9):
                ky, kx = tap // 3, tap % 3
                off = ky * W36 + kx + 1
                for kc in range(KC1):
                    for mc in range(MC):
                        nc.tensor.matmul(
                            ps1s[mc][:, :],
                            lhsT=w1_ts[tap][:, (kc * MC + mc) * P:
                                            (kc * MC + mc) * P + P],
                            rhs=x_t[:, kc * XBLK + off: kc * XBLK + off + N1],
                            start=(i_acc == 0), stop=(i_acc == n_acc - 1),
                        )
                    i_acc += 1
            for mc in range(MC):
                ps1 = ps1s[mc]
                src = bass.AP(ps1.tensor, ps1.offset,
                              [[N1, P], [W36, R1], [1, 32]])
                dstb = h1_t[:, :]
                dst = bass.AP(dstb.tensor, dstb.offset + mc * H1BLK + 2,
                              [[MC * H1BLK + H1SLACK, P], [W36, R1], [1, 32]])
                nc.scalar.activation(dst, src, Relu)

            # ---- conv2: 256 -> 256 over 5 rows --------------------------
            ps2s = [psp.tile([P, N1], f32, tag="cva", name="ps2a"),
                    psp.tile([P, N1], f32, tag="cvb", name="ps2b")]
            nacc2 = 9 * KC2
            iacc2 = 0
            for tap in range(9):
                ky, kx = tap // 3, tap % 3
                off = ky * W36 + kx + 1
                for kc in range(KC2):
                    for mc in range(MC):
                        nc.tensor.matmul(
                            ps2s[mc][:, 0:N2],
                            lhsT=w2_ts[tap // 3][:,
                                ((tap % 3) * KC2 + kc) * MC * P + mc * P:
                                ((tap % 3) * KC2 + kc) * MC * P + mc * P + P],
                            rhs=h1_t[:, kc * H1BLK + off:
                                     kc * H1BLK + off + N2],
                            start=(iacc2 == 0), stop=(iacc2 == nacc2 - 1),
                        )
                    iacc2 += 1
            for mc in range(MC):
                ps2 = ps2s[mc]
                src2 = bass.AP(ps2.tensor, ps2.offset,
                               [[N1, P], [W36, R2], [1, 32]])
                h2b_ = h2_t[:, :]
                dst2 = bass.AP(h2b_.tensor, h2b_.offset + mc * 160,
                               [[MC * 160, P], [32, R2], [1, 32]])
                nc.scalar.activation(dst2, src2, Relu, bias=b2_t[:, mc:mc + 1])

            # ---- 1x1 conv -> psr cols (24r + c), r-major contiguous -----
            psr = psp.tile([32, 120], f32, tag="psr")
            psrb = psr[:, :]
            rtb = rt_t[:, :]
            for r in range(R2):
                for kc in range(KC2):
                    nc.tensor.matmul(
                        psr[:, r * OUT_C:(r + 1) * OUT_C],
                        lhsT=h2_t[:, kc * 160 + r * 32: kc * 160 + r * 32 + 32],
                        rhs=wr_t[:, kc * OUT_C:(kc + 1) * OUT_C],
                        start=(kc == 0), stop=(kc == KC2 - 1),
                    )
            # psr -> rt (fp32 -> fp16)
            nc.vector.tensor_copy(
                bass.AP(rtb.tensor, rtb.offset, [[120, 32], [1, 120]]),
                bass.AP(psrb.tensor, psrb.offset, [[120, 32], [1, 120]]))

            # ---- W expansion (all 4 groups, M=96), bias via rt row 32 ---
            psw = psp.tile([96, 1024], f32, tag="psw")
            psa = psp.tile([96, 1024], f32, tag="psa")
            lhs_p = bass.AP(rtb.tensor, rtb.offset, [[120, 33], [1, 96]])
            lhs_n = bass.AP(rtb.tensor, rtb.offset + OUT_C, [[120, 32], [1, 96]])
            for j in range(2):
                nc.tensor.matmul(psw[:, j * 512:(j + 1) * 512],
                                 lhsT=lhs_p,
                                 rhs=em_t[:, j * 512:(j + 1) * 512],
                                 start=True, stop=True)
                nc.tensor.matmul(psa[:, j * 512:(j + 1) * 512],
                                 lhsT=lhs_p,
                                 rhs=em_t[:, 1024 + j * 512:
                                          1024 + (j + 1) * 512],
                                 start=True, stop=False)
                nc.tensor.matmul(psa[:, j * 512:(j + 1) * 512],
                                 lhsT=lhs_n,
                                 rhs=em_t[0:32, 1024 + j * 512:
                                          1024 + (j + 1) * 512],
                                 start=False, stop=True)
            # PSUM -> SBUF: [pure | pure | avg]
            nc.scalar.activation(rw_t[:, 0:1024], psw[:, :], Ident)
            nc.vector.tensor_copy(rw_t[:, 1024:2048], psw[:, :])
            nc.vector.tensor_copy(rw_t[:, 2048:3072], psa[:, :])

            # ---- store: rows 0-19 as 2-row 4KB descs, rows 20-21 as 4KB -
            rwb_ = rw_t[:, :]
            nc.sync.dma_start(
                bass.AP(out_d, 0, [[RUN * OUT_W, 96], [2048, 5], [1, 2048]]),
                bass.AP(rwb_.tensor, rwb_.offset, [[3072, 96], [0, 5], [1, 2048]]))
            nc.scalar.dma_start(
                bass.AP(out_d, 10 * OUT_W,
                        [[RUN * OUT_W, 96], [2048, 5], [1, 2048]]),
                bass.AP(rwb_.tensor, rwb_.offset, [[3072, 96], [0, 5], [1, 2048]]))
            nc.gpsimd.dma_start(
                bass.AP(out_d, 20 * OUT_W, [[RUN * OUT_W, 96], [1, 2048]]),
                bass.AP(rwb_.tensor, rwb_.offset + 1024, [[3072, 96], [1, 2048]]))

    nc.compile()
    return nc


def _pack_inputs(x, w1, b1, w2, b2, wr, br):
    x = np.asarray(x, np.float32)
    w1 = np.asarray(w1, np.float32)
    w2 = np.asarray(w2, np.float32)
    wr = np.asarray(wr, np.float32)
    b1 = np.asarray(b1, np.float32)
    b2 = np.asarray(b2, np.float32)
    br = np.asarray(br, np.float32)

    xp = np.zeros((NCORES, P, KC1, RX, W36), np.float16)
    xv = x[0]  # (576, 32, 32)
    for k in range(NCORES):
        for r in range(RX):
            g = 4 * k - 2 + r
            if 0 <= g < H:
                blkv = xv[:, g, :]  # (576, 32)
                xp[k, :, :4, r, 2:34] = blkv[:512].reshape(4, P, W).transpose(1, 0, 2)
                xp[k, :64, 4, r, 2:34] = blkv[512:]
                xp[k, 64, 4, r, 2:34] = 1.0  # valid-row indicator (bias)
            else:
                # invalid-row indicator: forces h1 row r-1 below zero so
                # ReLU clamps it to exactly 0 (replaces the row mask)
                xp[k, 65, 4, r, 2:34] = 1.0
    xp = xp.reshape(NCORES, P, KC1 * XBLK)
    xp = np.concatenate([xp, np.zeros((NCORES, P, XSLACK), np.float16)], axis=2)

    # w1: [p, tap, kc, mc, m] = w1[mc*128+m, kc*128+p, ky, kx]
    w1p = np.zeros((P, 9, KC1, MC, P), np.float16)
    w1v = w1.transpose(2, 3, 1, 0).reshape(9, IN_C, MID_C)  # (tap, ci, co)
    w1p[:, :, :4, :, :] = (
        w1v[:, :512, :].reshape(9, 4, P, MC, P).transpose(2, 0, 1, 3, 4))
    w1p[:64, :, 4, :, :] = w1v[:, 512:, :].reshape(9, 64, MC, P).transpose(1, 0, 2, 3)
    # bias enters via the indicator channel on the center tap; the
    # invalid-row indicator drives out-of-image h1 rows to relu(-1000)=0
    w1p[64, 4, 4, :, :] = b1.reshape(MC, P)
    w1p[65, 4, 4, :, :] = -1000.0
    w1p = w1p.reshape(P, 9 * KC1 * MC * P)

    w2p = np.zeros((P, 9, KC2, MC, P), np.float16)
    w2v = w2.transpose(2, 3, 1, 0).reshape(9, MID_C, MID_C)
    w2p[:, :, :, :, :] = (
        w2v.reshape(9, KC2, P, MC, P).transpose(2, 0, 1, 3, 4))
    w2p = w2p.reshape(P, 9 * KC2 * MC * P)

    wrp = wr.T.reshape(KC2, P, OUT_C).transpose(1, 0, 2).reshape(P, KC2 * OUT_C)
    wrp = np.ascontiguousarray(wrp, np.float16)
    b2p = b2.reshape(MC, P).T.copy()

    # em: [33, 2048]: cols 0-1023 = E with bias row 1, cols 1024-2047 = E/2
    # with bias row 1 (bias added once; the lhs_n matmul excludes row 32)
    E = (np.arange(OUT_W) // 32 == np.arange(32)[:, None]).astype(np.float16)
    em = np.zeros((33, 2048), np.float16)
    em[:32, :1024] = E
    em[:32, 1024:] = 0.5 * E
    em[32, :] = 1.0

    # bias row for rt (free idx 24r + c -> br[c])
    brow = np.tile(br, 5).reshape(1, 120).astype(np.float16)

    shared = dict(w1p=w1p, w2p=w2p, wrp=wrp, b2p=b2p, emp=em, browp=brow)
    in_maps = []
    for k in range(NCORES):
        m = dict(shared)
        m["xs"] = np.ascontiguousarray(xp[k])
        in_maps.append(m)
    return in_maps


def kernel(x, w1, b1, w2, b2, wr, br):
    from concourse.bass_utils import run_bass_kernel_spmd

    if "nc" not in _prog_cache:
        _prog_cache["nc"] = _build_program()
    nc = _prog_cache["nc"]

    in_maps = _pack_inputs(x, w1, b1, w2, b2, wr, br)
    res = run_bass_kernel_spmd(nc, in_maps, list(range(NCORES)))

    _, t = _h_runs()
    out = np.empty((1, OUT_C, OUT_H, OUT_W), np.float32)
    for k in range(NCORES):
        # outb rows: m = hl*24 + c ; cols: RUN x 1024
        buf = res.results[k]["outb"].astype(np.float32)
        buf = buf.reshape(4, OUT_C, RUN, OUT_W)
        for hl in range(4):
            h = 4 * k + hl
            n = t[h + 1] - t[h]
            g = buf[hl]  # (OUT_C, RUN, OUT_W)
            out[0, :, t[h]:t[h] + n - 1, :] = g[:, 0:n - 1, :]
            # last row of the run: averaged row (except global last row,
            # where the run is pure replication and n-1 <= 20)
            if h < H - 1:
                out[0, :, t[h] + n - 1, :] = g[:, RUN - 1, :]
            else:
                out[0, :, t[h] + n - 1, :] = g[:, n - 1, :]
    return out


# revision 19
# speedup vs baseline: 1.2111x; 1.0449x over previous
"""Trainium2 Bass kernel for nn_ConvProjector (conv3x3 -> ReLU -> conv3x3 -> ReLU
-> adaptive-avg-pool upsample 32x32 -> 687x1024 -> 1x1 conv 256->24 + bias).

v1 structure (vs v0 baseline):
  * 3 DMA queues (sync HWDGE, scalar HWDGE, gpsimd SWDGE) for weight
    streaming and output writes.
  * conv1 bias + out-of-image row masking folded into the matmul via an
    indicator channel in the spare kc4 partition (row 64): psum already
    holds conv+bias, and pad rows come out exactly 0 after plain ReLU.
  * 1x1-conv bias folded into the W-expansion matmul via a 33rd lhsT
    partition holding br per output column.
  * Expanded tile laid out [pure | pure | avg] (48 x 3072 per half) so the
    H-replication DMA uses 4KB contiguous descriptors (2 output rows per
    descriptor); rows 20-21 ship as one 4KB-desc DMA from offset 1024.
  * conv2 + 1x1 + expansion + store split into two halves (h2 rows 0-2 /
    3-4) so half B's compute overlaps half A's output DMA.
  * Sharding: unchanged - core k owns input rows 4k..4k+3, channel-complete
    convs on a 9-row x slice, no collectives.
"""
import sys

if '/opt/trn_rl_repo' not in sys.path:
    sys.path.insert(0, '/opt/trn_rl_repo')

import numpy as np

IN_C, MID_C, OUT_C = 576, 256, 24
H = W = 32
OUT_H, OUT_W = 687, 1024
NCORES = 8
P = 128
KC1 = 5           # ceil(576/128) input-channel chunks for conv1 (padded to 640)
KC2 = 2           # 256/128 chunks for conv2 / 1x1
MC = 2            # 256/128 output-channel chunks for conv1/conv2
W36 = 36          # padded row width (2 zero cols each side)
RX, R1, R2 = 9, 7, 5          # x rows / h1 rows / h2 (=r) rows per core
XBLK = RX * W36               # 324  per-kc x block
XSLACK = 16                   # rhs overrun slack
N1 = 256                      # conv1 matmul N (valid span 252)
H1BLK = R1 * W36              # 252  per-mc h1 block
H1SLACK = 80
N2 = 192                      # conv2 matmul N (valid span 176)
RUN = 22                      # output rows per owned input row

_prog_cache = {}


def _h_runs():
    i = np.arange(OUT_H)
    s = (i * H) // OUT_H
    t = np.searchsorted(s, np.arange(H + 1), side='left')
    return s, t


def _build_program():
    import concourse.bass as bass
    import concourse.bacc as bacc
    import concourse.mybir as mybir
    from concourse.tile import TileContext

    f32 = mybir.dt.float32
    f16 = mybir.dt.float16
    nc = bacc.Bacc("TRN2", target_bir_lowering=False, debug=False,
                   num_devices=NCORES)

    xs_d = nc.dram_tensor("xs", [P, KC1 * XBLK + XSLACK], f16, kind="ExternalInput")
    w1_d = nc.dram_tensor("w1p", [P, 9 * KC1 * MC * P], f16, kind="ExternalInput")
    w2_d = nc.dram_tensor("w2p", [P, 9 * KC2 * MC * P], f16, kind="ExternalInput")
    wr_d = nc.dram_tensor("wrp", [P, KC2 * OUT_C], f16, kind="ExternalInput")
    b2_d = nc.dram_tensor("b2p", [P, MC], f32, kind="ExternalInput")
    em_d = nc.dram_tensor("emp", [33, 2048], f16, kind="ExternalInput")
    br_d = nc.dram_tensor("browp", [1, 120], f16, kind="ExternalInput")
    out_d = nc.dram_tensor("outb", [96, RUN * OUT_W], f16, kind="ExternalOutput")

    Relu = mybir.ActivationFunctionType.Relu
    Ident = mybir.ActivationFunctionType.Identity
    w1blk = KC1 * MC * P          # 1280 elems per tap row
    w2blk = 3 * KC2 * MC * P      # 1536 elems per 3-tap group row

    with TileContext(nc) as tc:
        with (
            tc.tile_pool(name="sb", bufs=1) as sb,
            tc.tile_pool(name="ps", bufs=1, space="PSUM") as psp,
        ):
            x_t = sb.tile([P, KC1 * XBLK + XSLACK], f16)
            w1_ts = [sb.tile([P, w1blk], f16, tag=f"w1_{t}", name=f"w1t{t}")
                     for t in range(9)]
            w2_t = sb.tile([P, 9 * KC2 * MC * P], f16)
            wr_t = sb.tile([P, KC2 * OUT_C], f16)
            b2_t = sb.tile([P, MC], f32)
            em_t = sb.tile([33, 2048], f16)
            h1_t = sb.tile([P, MC * H1BLK + H1SLACK], f16)
            h2_t = sb.tile([P, MC * 160], f16)
            rt_t = sb.tile([33, 120], f16)
            rw_t = sb.tile([96, 3072], f16)
            # spare-partition duplicate: partition 96+s holds the [p|p|avg]
            # patterns of combos 3s, 3s+1, 3s+2 (spreads the output DMA's
            # SBUF-port load over all 128 partitions)
            rwd_t = sb.tile([P, 3 * 3072], f16)

            # ---- input loads: 3 queues, taps arrive roughly in order ----
            def tap_dma(eng, t):
                eng.dma_start(
                    w1_ts[t][:],
                    bass.AP(w1_d, t * w1blk, [[9 * w1blk, P], [1, w1blk]]))

            # first wave: t0 / x / t2 each alone on a queue, then FIFO
            tap_dma(nc.sync, 0)
            nc.scalar.dma_start(x_t[:], xs_d.ap())
            tap_dma(nc.gpsimd, 2)
            tap_dma(nc.sync, 1)
            tap_dma(nc.scalar, 3)
            tap_dma(nc.gpsimd, 6)
            tap_dma(nc.sync, 4)
            tap_dma(nc.scalar, 5)
            tap_dma(nc.gpsimd, 8)
            tap_dma(nc.sync, 7)
            nc.gpsimd.dma_start(w2_t[:], w2_d.ap())
            nc.scalar.dma_start(wr_t[:], wr_d.ap())
            nc.scalar.dma_start(b2_t[:], b2_d.ap())
            nc.scalar.dma_start(em_t[:], em_d.ap())
            nc.gpsimd.dma_start(rt_t[32:33, :], br_d.ap())

            # h1 pad cols must be zero; ReLU only writes valid 32-col spans.
            nc.vector.memset(h1_t[:], 0.0)

            # ---- conv1: 576 -> 256 over 7 rows, bias via indicator ------
            ps1s = [psp.tile([P, N1], f32, tag="cva", name="ps1a"),
                    psp.tile([P, N1], f32, tag="cvb", name="ps1b")]
            n_acc = 9 * KC1
            i_acc = 0
            for tap in range(9):
                ky, kx = tap // 3, tap % 3
                off = ky * W36 + kx + 1
                for kc in range(KC1):
                    for mc in range(MC):
                        nc.tensor.matmul(
                            ps1s[mc][:, :],
                            lhsT=w1_ts[tap][:, (kc * MC + mc) * P:
                                            (kc * MC + mc) * P + P],
                            rhs=x_t[:, kc * XBLK + off: kc * XBLK + off + N1],
                            start=(i_acc == 0), stop=(i_acc == n_acc - 1),
                        )
                    i_acc += 1
            for mc in range(MC):
                ps1 = ps1s[mc]
                src = bass.AP(ps1.tensor, ps1.offset,
                              [[N1, P], [W36, R1], [1, 32]])
                dstb = h1_t[:, :]
                dst = bass.AP(dstb.tensor, dstb.offset + mc * H1BLK + 2,
                              [[MC * H1BLK + H1SLACK, P], [W36, R1], [1, 32]])
                nc.scalar.activation(dst, src, Relu)

            # ---- conv2: 256 -> 256 over 5 rows --------------------------
            ps2s = [psp.tile([P, N1], f32, tag="cva", name="ps2a"),
                    psp.tile([P, N1], f32, tag="cvb", name="ps2b")]
            nacc2 = 9 * KC2
            iacc2 = 0
            for tap in range(9):
                ky, kx = tap // 3, tap % 3
                off = ky * W36 + kx + 1
                for kc in range(KC2):
                    for mc in range(MC):
                        nc.tensor.matmul(
                            ps2s[mc][:, 0:N2],
                            lhsT=w2_t[:, ((tap * KC2 + kc) * MC + mc) * P:
                                      ((tap * KC2 + kc) * MC + mc) * P + P],
                            rhs=h1_t[:, kc * H1BLK + off:
                                     kc * H1BLK + off + N2],
                            start=(iacc2 == 0), stop=(iacc2 == nacc2 - 1),
                        )
                    iacc2 += 1
            for mc in range(MC):
                ps2 = ps2s[mc]
                src2 = bass.AP(ps2.tensor, ps2.offset,
                               [[N1, P], [W36, R2], [1, 32]])
                h2b_ = h2_t[:, :]
                dst2 = bass.AP(h2b_.tensor, h2b_.offset + mc * 160,
                               [[MC * 160, P], [32, R2], [1, 32]])
                nc.scalar.activation(dst2, src2, Relu, bias=b2_t[:, mc:mc + 1])

            # ---- 1x1 conv -> psr cols (24r + c), r-major contiguous -----
            psr = psp.tile([32, 120], f32, tag="psr")
            psrb = psr[:, :]
            rtb = rt_t[:, :]
            for r in range(R2):
                for kc in range(KC2):
                    nc.tensor.matmul(
                        psr[:, r * OUT_C:(r + 1) * OUT_C],
                        lhsT=h2_t[:, kc * 160 + r * 32: kc * 160 + r * 32 + 32],
                        rhs=wr_t[:, kc * OUT_C:(kc + 1) * OUT_C],
                        start=(kc == 0), stop=(kc == KC2 - 1),
                    )
            # psr -> rt (fp32 -> fp16)
            nc.vector.tensor_copy(
                bass.AP(rtb.tensor, rtb.offset, [[120, 32], [1, 120]]),
                bass.AP(psrb.tensor, psrb.offset, [[120, 32], [1, 120]]))

            # ---- W expansion (all 4 groups, M=96), bias via rt row 32 ---
            psw = psp.tile([96, 1024], f32, tag="psw")
            psa = psp.tile([96, 1024], f32, tag="psa")
            lhs_p = bass.AP(rtb.tensor, rtb.offset, [[120, 33], [1, 96]])
            lhs_n = bass.AP(rtb.tensor, rtb.offset + OUT_C, [[120, 32], [1, 96]])
            for j in range(2):
                nc.tensor.matmul(psw[:, j * 512:(j + 1) * 512],
                                 lhsT=lhs_p,
                                 rhs=em_t[:, j * 512:(j + 1) * 512],
                                 start=True, stop=True)
                nc.tensor.matmul(psa[:, j * 512:(j + 1) * 512],
                                 lhsT=lhs_p,
                                 rhs=em_t[:, 1024 + j * 512:
                                          1024 + (j + 1) * 512],
                                 start=True, stop=False)
                nc.tensor.matmul(psa[:, j * 512:(j + 1) * 512],
                                 lhsT=lhs_n,
                                 rhs=em_t[0:32, 1024 + j * 512:
                                          1024 + (j + 1) * 512],
                                 start=False, stop=True)
            # PSUM -> SBUF: [pure | pure | avg] on three engines in parallel
            nc.scalar.activation(rw_t[:, 0:1024], psw[:, :], Ident)
            nc.vector.tensor_copy(rw_t[:, 1024:2048], psw[:, :])
            nc.vector.tensor_copy(rw_t[:, 2048:3072], psa[:, :])

            # ---- store ----------------------------------------------------
            # partitions 0-95 ship rows 0-16; spare partitions 96-127 (3
            # combos each, via dup DMAs) ship rows 17-21. This spreads the
            # per-partition SBUF port load (~2.8 GB/s/partition) across all
            # 128 partitions.
            rwb_ = rw_t[:, :]
            rwdb = rwd_t[:, :]
            # dup_t: combos t, 3+t, ..., 93+t -> partition 96+s col t*3072
            for tdup, eng in enumerate((nc.sync, nc.scalar, nc.gpsimd)):
                eng.dma_start(
                    bass.AP(rwdb.tensor, rwdb.offset + 96 * 3 * 3072 + tdup * 3072,
                            [[3 * 3072, 32], [1, 3072]]),
                    bass.AP(rwb_.tensor, rwb_.offset + tdup * 3072,
                            [[3 * 3072, 32], [1, 3072]]))
            # mains: rows 0-15 (2-row 4KB descs) + row 16 (2KB descs)
            nc.sync.dma_start(
                bass.AP(out_d, 0, [[RUN * OUT_W, 96], [2048, 4], [1, 2048]]),
                bass.AP(rwb_.tensor, rwb_.offset, [[3072, 96], [0, 4], [1, 2048]]))
            nc.scalar.dma_start(
                bass.AP(out_d, 8 * OUT_W,
                        [[RUN * OUT_W, 96], [2048, 4], [1, 2048]]),
                bass.AP(rwb_.tensor, rwb_.offset, [[3072, 96], [0, 4], [1, 2048]]))
            nc.gpsimd.dma_start(
                bass.AP(out_d, 16 * OUT_W, [[RUN * OUT_W, 96], [1, 1024]]),
                bass.AP(rwb_.tensor, rwb_.offset, [[3072, 96], [1, 1024]]))
            # spares: rows 17-18, 19-20 (pure), 21 (avg)
            sp_src = rwdb.offset + 96 * 3 * 3072
            nc.scalar.dma_start(
                bass.AP(out_d, 17 * OUT_W,
                        [[3 * RUN * OUT_W, 32], [RUN * OUT_W, 3], [1, 2048]]),
                bass.AP(rwdb.tensor, sp_src, [[3 * 3072, 32], [3072, 3], [1, 2048]]))
            nc.sync.dma_start(
                bass.AP(out_d, 19 * OUT_W,
                        [[3 * RUN * OUT_W, 32], [RUN * OUT_W, 3], [1, 2048]]),
                bass.AP(rwdb.tensor, sp_src, [[3 * 3072, 32], [3072, 3], [1, 2048]]))
            nc.gpsimd.dma_start(
                bass.AP(out_d, 21 * OUT_W,
                        [[3 * RUN * OUT_W, 32], [RUN * OUT_W, 3], [1, 1024]]),
                bass.AP(rwdb.tensor, sp_src + 2048,
                        [[3 * 3072, 32], [3072, 3], [1, 1024]]))

    nc.compile()
    return nc


def _pack_inputs(x, w1, b1, w2, b2, wr, br):
    x = np.asarray(x, np.float32)
    w1 = np.asarray(w1, np.float32)
    w2 = np.asarray(w2, np.float32)
    wr = np.asarray(wr, np.float32)
    b1 = np.asarray(b1, np.float32)
    b2 = np.asarray(b2, np.float32)
    br = np.asarray(br, np.float32)

    xp = np.zeros((NCORES, P, KC1, RX, W36), np.float16)
    xv = x[0]  # (576, 32, 32)
    for k in range(NCORES):
        for r in range(RX):
            g = 4 * k - 2 + r
            if 0 <= g < H:
                blkv = xv[:, g, :]  # (576, 32)
                xp[k, :, :4, r, 2:34] = blkv[:512].reshape(4, P, W).transpose(1, 0, 2)
                xp[k, :64, 4, r, 2:34] = blkv[512:]
                xp[k, 64, 4, r, 2:34] = 1.0  # valid-row indicator (bias)
            else:
                # invalid-row indicator: forces h1 row r-1 below zero so
                # ReLU clamps it to exactly 0 (replaces the row mask)
                xp[k, 65, 4, r, 2:34] = 1.0
    xp = xp.reshape(NCORES, P, KC1 * XBLK)
    xp = np.concatenate([xp, np.zeros((NCORES, P, XSLACK), np.float16)], axis=2)

    # w1: [p, tap, kc, mc, m] = w1[mc*128+m, kc*128+p, ky, kx]
    w1p = np.zeros((P, 9, KC1, MC, P), np.float16)
    w1v = w1.transpose(2, 3, 1, 0).reshape(9, IN_C, MID_C)  # (tap, ci, co)
    w1p[:, :, :4, :, :] = (
        w1v[:, :512, :].reshape(9, 4, P, MC, P).transpose(2, 0, 1, 3, 4))
    w1p[:64, :, 4, :, :] = w1v[:, 512:, :].reshape(9, 64, MC, P).transpose(1, 0, 2, 3)
    # bias enters via the indicator channel on the center tap; the
    # invalid-row indicator drives out-of-image h1 rows to relu(-1000)=0
    w1p[64, 4, 4, :, :] = b1.reshape(MC, P)
    w1p[65, 4, 4, :, :] = -1000.0
    w1p = w1p.reshape(P, 9 * KC1 * MC * P)

    w2p = np.zeros((P, 9, KC2, MC, P), np.float16)
    w2v = w2.transpose(2, 3, 1, 0).reshape(9, MID_C, MID_C)
    w2p[:, :, :, :, :] = (
        w2v.reshape(9, KC2, P, MC, P).transpose(2, 0, 1, 3, 4))
    w2p = w2p.reshape(P, 9 * KC2 * MC * P)

    wrp = wr.T.reshape(KC2, P, OUT_C).transpose(1, 0, 2).reshape(P, KC2 * OUT_C)
    wrp = np.ascontiguousarray(wrp, np.float16)
    b2p = b2.reshape(MC, P).T.copy()

    # em: [33, 2048]: cols 0-1023 = E with bias row 1, cols 1024-2047 = E/2
    # with bias row 1 (bias added once; the lhs_n matmul excludes row 32)
    E = (np.arange(OUT_W) // 32 == np.arange(32)[:, None]).astype(np.float16)
    em = np.zeros((33, 2048), np.float16)
    em[:32, :1024] = E
    em[:32, 1024:] = 0.5 * E
    em[32, :] = 1.0

    # bias row for rt (free idx 24r + c -> br[c])
    brow = np.tile(br, 5).reshape(1, 120).astype(np.float16)

    shared = dict(w1p=w1p, w2p=w2p, wrp=wrp, b2p=b2p, emp=em, browp=brow)
    in_maps = []
    for k in range(NCORES):
        m = dict(shared)
        m["xs"] = np.ascontiguousarray(xp[k])
        in_maps.append(m)
    return in_maps


def kernel(x, w1, b1, w2, b2, wr, br):
    from concourse.bass_utils import run_bass_kernel_spmd

    if "nc" not in _prog_cache:
        _prog_cache["nc"] = _build_program()
    nc = _prog_cache["nc"]

    in_maps = _pack_inputs(x, w1, b1, w2, b2, wr, br)
    res = run_bass_kernel_spmd(nc, in_maps, list(range(NCORES)))

    _, t = _h_runs()
    out = np.empty((1, OUT_C, OUT_H, OUT_W), np.float32)
    for k in range(NCORES):
        # outb rows: m = hl*24 + c ; cols: RUN x 1024
        buf = res.results[k]["outb"].astype(np.float32)
        buf = buf.reshape(4, OUT_C, RUN, OUT_W)
        for hl in range(4):
            h = 4 * k + hl
            n = t[h + 1] - t[h]
            g = buf[hl]  # (OUT_C, RUN, OUT_W)
            out[0, :, t[h]:t[h] + n - 1, :] = g[:, 0:n - 1, :]
            # last row of the run: averaged row (except global last row,
            # where the run is pure replication and n-1 <= 20)
            if h < H - 1:
                out[0, :, t[h] + n - 1, :] = g[:, RUN - 1, :]
            else:
                out[0, :, t[h] + n - 1, :] = g[:, n - 1, :]
    return out
